# revision 69
# baseline (speedup 1.0000x reference)
"""NetVLAD Trainium2 kernel v2 — data-parallel over N across 8 cores.

Per core: 4 images [C=128, P=4096], processed in 4 chunks of 1024 px
(8 px-tiles of 128). Host ships x twice in bf16: normal layout [C, P]
(logits lhsT) and a pre-shuffled transposed layout
[img, chunk, pixel%128, tile, C] so the vlad rhs xT tiles arrive as
full-bandwidth contiguous DMA (2 KB/partition rows) with no on-chip
transpose or PSUM eviction.

Math per chunk, layout [pixel-partition, free]:
  PE:   u[p,(t,k)] = x_t.T @ wT  (8 bf16 matmuls, t-major PSUM)
        ssq rows: ones.T @ xsq_t -> psumS[8,128]; sqrt -> nrowS (fp32r)
        rank-1:  u += nrowS.T @ b_blockdiag  (one fp32r matmul, N=512)
        => psumL holds u~ = w@x + n*b  (fp32 exact)
  DVE:  m~ = max_k u~ (per tile); tiny stats (1/n, -m~/n, rcol)
  ACT:  per-tile exp: ee = Exp(u~ * invn + (-m~*invn)) via per-partition
        scale/bias APs, fp32-exact from PSUM, bf16 out (k-major strided)
  DVE:  sumexp; rcol = invn/sum (bf16)
  DVE/POOL: aa = ee * rcol  (bf16 2x broadcast, k-major)
  PE:   vlad[56,128] += aa_t.T @ xT_t ; s_row[1,56] += ncol_t.T @ aa_t
Tail per image: vlad = t1 - s*cen, intra-norm over k (PE transpose),
global L2, write [56,128].
"""

import sys

for _p in ("/opt/trn_rl_repo",):
    if _p not in sys.path:
        sys.path.insert(0, _p)

import numpy as np

NIMG = 4      # images per core
C = 128
K = 64
KE = 56
P = 4096
TPC = 8       # pixel tiles (128 px) per chunk
CH = TPC * 128
NCH = P // CH  # 4 chunks per image

_cache = {}


def _build():
    import concourse.bass as bass
    import concourse.mybir as mybir
    from concourse import bacc, tile

    f32 = mybir.dt.float32
    f32r = mybir.dt.float32r
    bf16 = mybir.dt.bfloat16
    Alu = mybir.AluOpType
    Act = mybir.ActivationFunctionType
    Ax = mybir.AxisListType

    nc = bacc.Bacc()
    # normal layout, bf16
    xn_in = nc.declare_dram_parameter("xn", [NIMG, C, P], bf16, isOutput=False)
    # shuffled transpose: [img, p(128), chunk*tile*c] (pixel-major)
    xt_in = nc.declare_dram_parameter("xt", [NIMG, 128, NCH * TPC * C], bf16,
                                      isOutput=False)
    # consts, bf16 block: wT[0:64] | ident[64:192] | onesc[192:193]
    cstb_in = nc.declare_dram_parameter("cstb", [C, 200], bf16, isOutput=False)
    # consts, f32 block: cen (partitions 0:56, cols 0:128) | b-blockdiag
    # (partitions 0:8, cols 128:640) | identity f32 (cols 640:768)
    cstf_in = nc.declare_dram_parameter("cstf", [C, 936], f32, isOutput=False)
    out_ext = nc.declare_dram_parameter("out", [NIMG, KE, C], f32, isOutput=True)

    with tile.TileContext(nc) as tc:
        with (
            tc.tile_pool(name="const", bufs=1) as cpool,
            tc.tile_pool(name="xin", bufs=3) as xpool,
            tc.tile_pool(name="work", bufs=3) as wpool,
            tc.tile_pool(name="stats", bufs=2) as spool,
            tc.tile_pool(name="fin", bufs=3) as fpool,
            tc.tile_pool(name="psL", bufs=3, space="PSUM") as pL,
            tc.tile_pool(name="psS", bufs=1, space="PSUM") as pS,
            tc.tile_pool(name="psN", bufs=1, space="PSUM") as pN,
            tc.tile_pool(name="psV", bufs=1, space="PSUM") as pV,
            tc.tile_pool(name="psS2", bufs=1, space="PSUM") as pS2,
            tc.tile_pool(name="psM", bufs=1, space="PSUM") as pM,
        ):
            def emit_rsqrt(eng, pool, shape, out, in_, tag, iters=2):
                """out = 1/sqrt(in_) via magic-seed Newton (SBUF fp32 only)."""
                i32 = mybir.dt.int32
                t1 = pool.tile(shape, i32, tag=tag + "t1")
                eng.tensor_scalar(t1[:], in_.bitcast(i32), 1, -1,
                                  Alu.logical_shift_right, Alu.bitwise_xor)
                t2 = pool.tile(shape, i32, tag=tag + "t2")
                eng.tensor_scalar(t2[:], t1[:], 0x5f3759df + 1, None, Alu.add)
                cur = t2[:].bitcast(f32)
                for it in range(iters):
                    last = it == iters - 1
                    u = pool.tile(shape, f32, tag=tag + f"u{it}")
                    eng.tensor_tensor(u[:], cur, cur, Alu.mult)
                    v = pool.tile(shape, f32, tag=tag + f"v{it}")
                    eng.scalar_tensor_tensor(v[:], u[:], -0.5, in_,
                                             Alu.mult, Alu.mult)
                    w = out if last else pool.tile(shape, f32,
                                                   tag=tag + f"w{it}")
                    eng.scalar_tensor_tensor(w[:], v[:], 1.5, cur,
                                             Alu.add, Alu.mult)
                    cur = w[:]

            def emit_rsqrt_pool(pool, shape, out, in_, tag, iters=2):
                """rsqrt: int seed on DVE, Newton iters as plain TTs on Pool."""
                i32 = mybir.dt.int32
                nhT = cstf[:, 900:900 + shape[1]]
                p15T = cstf[:, 916:916 + shape[1]]
                t1 = pool.tile(shape, i32, tag=tag + "t1")
                nc.vector.tensor_scalar(t1[:], in_.bitcast(i32), 1, -1,
                                        Alu.logical_shift_right,
                                        Alu.bitwise_xor)
                t2 = pool.tile(shape, i32, tag=tag + "t2")
                nc.vector.tensor_scalar(t2[:], t1[:], 0x5f3759df + 1, None,
                                        Alu.add)
                cur = t2[:].bitcast(f32)
                for it in range(iters):
                    last = it == iters - 1
                    u = pool.tile(shape, f32, tag=tag + f"u{it}")
                    nc.gpsimd.tensor_tensor(u[:], cur, cur, Alu.mult)
                    q = pool.tile(shape, f32, tag=tag + f"q{it}")
                    nc.gpsimd.tensor_tensor(q[:], u[:], in_, Alu.mult)
                    hh = pool.tile(shape, f32, tag=tag + f"h{it}")
                    nc.gpsimd.tensor_tensor(hh[:], q[:], nhT, Alu.mult)
                    g = pool.tile(shape, f32, tag=tag + f"g{it}")
                    nc.gpsimd.tensor_tensor(g[:], hh[:], p15T, Alu.add)
                    w = out if last else pool.tile(shape, f32,
                                                   tag=tag + f"w{it}")
                    nc.gpsimd.tensor_tensor(w[:], g[:], cur, Alu.mult)
                    cur = w[:]

            cstb = cpool.tile([C, 200], bf16, tag="cstb")
            nc.scalar.dma_start(cstb[:], cstb_in[:])
            cstf = cpool.tile([C, 936], f32, tag="cstf")
            nc.scalar.dma_start(cstf[:], cstf_in[:])
            wT = cstb[:, 0:K]                  # [C, 64] bf16
            ident = cstb[:, 64:64 + C]         # [C, C] bf16 identity
            onesc = cstb[:, 192:193]           # [C, 1] bf16 ones
            cen = cstf[0:KE, 0:C]              # [56, 128] f32 centroids
            b8dg = cstf[0:TPC, 128:128 + TPC * K]  # [8, 512] f32
            identF = cstf[:, 640:640 + C]      # [C, C] f32 identity
            onesFc = cstf[:, 768:769]          # [C, 1] f32 ones
            onesF11 = cstf[0:1, 768:769]       # [1, 1] f32 one
            onesFr = cstf[0:1, 772:772 + C]    # [1, C] f32 ones row
            ones11 = cstb[0:1, 192:193]        # [1,1] bf16 one

            # misc PSUM bank: tail scratch + warm-up targets
            misc = pM.tile([C, 512], f32, tag="M")
            # PE warm-up observer of the const-DMA semaphores.
            nc.tensor.matmul(misc[0:1, 208:209], onesc, onesc, start=True,
                             stop=True)
            nc.tensor.matmul(misc[0:1, 209:210], cstf[0:1, 0:1],
                             cstf[0:1, 0:1], start=True, stop=True)

            def emit_stats(img):
                """DMA image img; stats computed per image-half [C, 16]."""
                xin = xpool.tile([C, P], bf16, tag="x")
                for cc in range(NCH):
                    nc.sync.dma_start(xin[:, cc * CH:(cc + 1) * CH],
                                      xn_in[img, :, cc * CH:(cc + 1) * CH])
                xts = xpool.tile([C, NCH * TPC * C], bf16, tag="xts")
                HP = NCH * TPC * C // 2
                nc.sync.dma_start(xts[:, 0:HP], xt_in[img, :, 0:HP])
                nc.sync.dma_start(xts[:, HP:], xt_in[img, :, HP:])

                psumS = pS.tile([C, NCH * TPC], f32, tag="S")
                psumN = pN.tile([TPC, NCH * C], f32, tag="N")
                halves = []
                for h in range(2):
                    HW2 = NCH // 2 * TPC        # 16 px-tiles per half
                    xsq = wpool.tile([C, P // 2], bf16, tag=f"xsq{h}")
                    pSh = psumS[:, h * HW2:(h + 1) * HW2]
                    for cc in range(NCH // 2):
                        ccg = h * NCH // 2 + cc
                        nc.gpsimd.tensor_mul(xsq[:, cc * CH:(cc + 1) * CH],
                                             xin[:, ccg * CH:(ccg + 1) * CH],
                                             xin[:, ccg * CH:(ccg + 1) * CH])
                        for j in range(TPC):
                            jj = cc * TPC + j
                            nc.tensor.matmul(pSh[:, jj:jj + 1],
                                             xsq[:, jj * 128:(jj + 1) * 128],
                                             onesc, start=True, stop=True,
                                             skip_group_check=True)
                    ssqS = spool.tile([C, HW2], f32, tag=f"ssqS{h}")
                    nc.vector.tensor_copy(ssqS[:], pSh)
                    invcI = spool.tile([C, HW2], f32, tag=f"invcI{h}")
                    emit_rsqrt(nc.vector, spool, [C, HW2], invcI,
                               ssqS[:], f"rsA{h}")
                    ncolfI = spool.tile([C, HW2], f32, tag=f"ncolfI{h}")
                    nc.gpsimd.tensor_tensor(ncolfI[:], ssqS[:], invcI[:],
                                            Alu.mult)
                    ncolI = spool.tile([C, HW2], bf16, tag=f"ncolI{h}")
                    nc.gpsimd.tensor_copy(ncolI[:], ncolfI[:])
                    pNh = psumN[:, h * (NCH // 2) * C:(h + 1) * (NCH // 2) * C]
                    for cc in range(NCH // 2):
                        nc.tensor.transpose(pNh[:, cc * C:(cc + 1) * C],
                                            ncolfI[:, cc * TPC:(cc + 1) * TPC],
                                            identF[0:C, 0:C])
                    nrowS = spool.tile([TPC, NCH // 2 * C], f32, tag=f"nrow{h}")
                    if h == 0:
                        nc.vector.tensor_copy(nrowS[:], pNh)
                    else:
                        nc.scalar.activation(nrowS[:], pNh, Act.Copy)
                    halves.append((invcI, ncolI, nrowS))
                return xin, xts, halves

            def emit_chunk(st, ch, psV, psS2):
                xin, xts, halves = st
                invcI, ncolI, nrowS = halves[ch // (NCH // 2)]
                chh = ch % (NCH // 2)
                xc = xin[:, ch * CH:(ch + 1) * CH]
                invc = invcI[:, chh * TPC:(chh + 1) * TPC]
                ncol = ncolI[:, chh * TPC:(chh + 1) * TPC]

                psumL = pL.tile([C, TPC * K], f32, tag="L")
                for j in range(TPC):
                    nc.tensor.matmul(psumL[:, j * K:(j + 1) * K],
                                     xc[:, j * 128:(j + 1) * 128], wT,
                                     start=(j == 0), stop=False,
                                     skip_group_check=True)
                nc.tensor.matmul(psumL[:, 0:TPC * K],
                                 nrowS[0:TPC, chh * C:(chh + 1) * C],
                                 b8dg, start=False, stop=True,
                                 skip_group_check=True)

                l3 = psumL[:].rearrange("p (t k) -> p t k", k=K)
                mcol = spool.tile([C, TPC], f32, tag="mcol")
                nc.vector.tensor_reduce(mcol[:], l3, axis=Ax.X, op=Alu.max)
                negmi = spool.tile([C, TPC], f32, tag="negmi")
                nc.vector.scalar_tensor_tensor(negmi[:], mcol[:], -1.0,
                                               invc, Alu.mult, Alu.mult)

                ee = wpool.tile([C, TPC * K], bf16, tag="ee")
                ee3 = ee[:].rearrange("p (k t) -> p k t", t=TPC)
                for j in range(TPC):
                    nc.scalar.activation(
                        ee3[:, :, j], psumL[:, j * K:(j + 1) * K],
                        Act.Exp, bias=negmi[:, j:j + 1],
                        scale=invc[:, j:j + 1])

                # sum over k: one bf16 2x TT stage, then a half-width reduce
                es = wpool.tile([C, K // 2 * TPC], bf16, tag="es")
                kk = K // 2 * TPC
                nc.gpsimd.tensor_tensor(es[:], ee[:, 0:kk], ee[:, kk:],
                                        Alu.add)
                es2 = wpool.tile([C, K // 4 * TPC], bf16, tag="es2")
                k2 = K // 4 * TPC
                nc.gpsimd.tensor_tensor(es2[:], es[:, 0:k2], es[:, k2:],
                                        Alu.add)
                e3 = es2[:].rearrange("p (k t) -> p t k", t=TPC)
                scol = spool.tile([C, TPC], f32, tag="scol")
                nc.vector.tensor_reduce(scol[:], e3, axis=Ax.X, op=Alu.add)
                gcol = spool.tile([C, TPC], f32, tag="gcol")
                nc.vector.reciprocal(gcol[:], scol[:])
                rcol = spool.tile([C, TPC], bf16, tag="rcol")
                nc.gpsimd.tensor_tensor(rcol[:], invc, gcol[:], Alu.mult)
                aa = wpool.tile([C, TPC * K], bf16, tag="aa")
                nc.vector.tensor_tensor(
                    aa[:].rearrange("p (k t) -> p k t", t=TPC),
                    ee[:].rearrange("p (k t) -> p k t", t=TPC),
                    rcol[:].rearrange("p t -> p () t").broadcast_to(
                        [C, K, TPC]),
                    Alu.mult)

                first = ch == 0
                last = ch == NCH - 1
                xtc = xts[:, ch * TPC * C:(ch + 1) * TPC * C]
                for j in range(TPC):
                    aslc = aa[:].rearrange("p (k t) -> p t k", t=TPC)[:, j, 0:KE]

                    nc.tensor.matmul(psV[0:KE, 0:C], aslc,
                                     xtc[:, j * C:(j + 1) * C],
                                     start=(first and j == 0),
                                     stop=(last and j == TPC - 1),
                                     skip_group_check=True)
                    nc.tensor.matmul(psS2[0:1, 0:KE], ncol[:, j:j + 1],
                                     aslc,
                                     start=(first and j == 0),
                                     stop=(last and j == TPC - 1),
                                     skip_group_check=True)

            def emit_tail(img, psV, psS2):
                srow = spool.tile([1, KE], f32, tag="srow")
                nc.vector.tensor_copy(srow[:], psS2[0:1, 0:KE])
                psC = misc[:, 200:208]
                nc.tensor.matmul(psC[0:KE, 0:1], srow[:], onesF11,
                                 start=True, stop=True)
                negs = spool.tile([KE, 1], f32, tag="negs")
                nc.vector.tensor_scalar_mul(negs[:], psC[0:KE, 0:1], -1.0)
                vk = fpool.tile([KE, C], f32, tag="vk")
                nc.vector.scalar_tensor_tensor(vk[:], cen, negs[:],
                                               psV[0:KE, 0:C],
                                               Alu.mult, Alu.add)
                ps = misc[:, 0:192]
                nc.tensor.transpose(ps[:, 0:KE], vk[:], identF[0:KE, 0:KE])
                trash = fpool.tile([C, KE], f32, tag="trash")
                ssqk = spool.tile([C, 1], f32, tag="ssqk")
                nc.scalar.activation(trash[:], ps[:, 0:KE], Act.Square,
                                     accum_out=ssqk[:])
                ssqkc = spool.tile([C, 1], f32, tag="ssqkc")
                nc.vector.tensor_scalar_max(ssqkc[:], ssqk[:], 1e-24)
                invk = spool.tile([C, 1], f32, tag="invk")
                emit_rsqrt(nc.vector, spool, [C, 1], invk, ssqkc[:], "rsK")
                t2 = spool.tile([C, 1], f32, tag="t2")
                nc.vector.scalar_tensor_tensor(t2[:], ssqkc[:], invk[:],
                                               invk[:], Alu.mult, Alu.mult)
                tiny = misc[:, 192:200]
                nc.tensor.matmul(tiny[0:1, 0:1], t2[:], onesFc,
                                 start=True, stop=True)
                totc = spool.tile([1, 1], f32, tag="totc")
                nc.vector.tensor_scalar_max(totc[:], tiny[0:1, 0:1], 1e-24)
                fv = spool.tile([1, 1], f32, tag="fv")
                emit_rsqrt(nc.vector, spool, [1, 1], fv, totc[:], "rsT")
                nc.tensor.matmul(tiny[:, 2:3], onesFr, fv[:],
                                 start=True, stop=True)
                comb = spool.tile([C, 1], f32, tag="comb")
                nc.vector.tensor_tensor(comb[:], invk[:], tiny[:, 2:3],
                                        Alu.mult)
                vnT = fpool.tile([C, KE], f32, tag="vnT")
                nc.vector.tensor_scalar(vnT[:], ps[:, 0:KE], comb[:], None,
                                        Alu.mult)
                nc.tensor.transpose(ps[0:KE, 64:64 + C], vnT[:], identF)
                ob = fpool.tile([KE, C], f32, tag="ob")
                nc.scalar.activation(ob[:], ps[0:KE, 64:64 + C], Act.Copy)
                nc.sync.dma_start(out_ext[img], ob[:])

            # software-pipelined emission: stats(i+1) interleaves into
            # image i's chunk stream so PE/Pool never stall at boundaries
            st = emit_stats(0)
            for img in range(NIMG):
                psV = pV.tile([C, 160], f32, tag="psV")
                psS2 = pS2.tile([1, K], f32, tag="psS2")
                nxt = None
                for ch in range(NCH):
                    emit_chunk(st, ch, psV, psS2)
                    if ch == 0 and img + 1 < NIMG:
                        nxt = emit_stats(img + 1)
                emit_tail(img, psV, psS2)
                st = nxt

    nc.compile()
    return nc


def _get_nc():
    if "nc" not in _cache:
        _cache["nc"] = _build()
    return _cache["nc"]


def build_in_map(x, conv_w, conv_b, centroids, core):
    """Build the per-core input map. x: [32, C, P or H,W] fp32 full batch."""
    import ml_dtypes

    x = np.asarray(x, dtype=np.float32).reshape(-1, C, P)
    per = NIMG
    xs = x[core * per:(core + 1) * per]                       # [4, C, P]
    xb = xs.astype(ml_dtypes.bfloat16)                        # [4, C, P]
    # shuffled transpose: [img, p, chunk, tile, c] (pixel-major)
    xt = np.ascontiguousarray(
        xb.reshape(NIMG, C, NCH, TPC, 128).transpose(0, 4, 2, 3, 1)
    ).reshape(NIMG, 128, NCH * TPC * C)

    cstb = np.zeros((C, 200), dtype=ml_dtypes.bfloat16)
    cstb[:, 0:K] = conv_w.T.astype(ml_dtypes.bfloat16)
    cstb[:, 64:64 + C] = np.eye(C, dtype=np.float32)
    cstb[:, 192:193] = 1.0

    cstf = np.zeros((C, 936), dtype=np.float32)
    cstf[0:KE, 0:C] = centroids[:KE].astype(np.float32)
    # block-diagonal bias: b8dg[tau, t*K + k] = b[k] iff t == tau
    bdg = np.zeros((TPC, TPC * K), dtype=np.float32)
    for t in range(TPC):
        bdg[t, t * K:(t + 1) * K] = conv_b.astype(np.float32)
    cstf[0:TPC, 128:128 + TPC * K] = bdg
    cstf[:, 640:640 + C] = np.eye(C, dtype=np.float32)
    cstf[:, 768:769] = 1.0
    cstf[0:1, 772:772 + C] = 1.0
    cstf[:, 900:916] = -0.5
    cstf[:, 916:932] = 1.5

    return {"xn": np.ascontiguousarray(xb), "xt": xt,
            "cstb": cstb, "cstf": cstf}


def kernel(x, conv_w, conv_b, centroids):
    from concourse.bass_utils import run_bass_kernel_spmd

    x = np.asarray(x, dtype=np.float32)
    conv_w = np.asarray(conv_w, dtype=np.float32)
    conv_b = np.asarray(conv_b, dtype=np.float32)
    centroids = np.asarray(centroids, dtype=np.float32)

    N = x.shape[0]
    n_cores = 8
    assert N // n_cores == NIMG

    in_maps = [build_in_map(x, conv_w, conv_b, centroids, i)
               for i in range(n_cores)]

    nc = _get_nc()
    res = run_bass_kernel_spmd(nc, in_maps, list(range(n_cores)))
    outs = [np.asarray(r["out"]).reshape(NIMG, KE * C) for r in res.results]
    return np.concatenate(outs, axis=0).astype(np.float32)


if __name__ == "__main__":
    rng = np.random.default_rng(0)
    x = rng.standard_normal((32, C, 64, 64), dtype=np.float32)
    w = rng.standard_normal((K, C), dtype=np.float32)
    b = rng.standard_normal((K,), dtype=np.float32)
    c = rng.random((K, C), dtype=np.float32)
    out = kernel(x=x, conv_w=w, conv_b=b, centroids=c)
    print(out.shape, out.dtype)
